# revision 37
# baseline (speedup 1.0000x reference)
"""CSWin-style cross-attention block for Trainium2 (Bass/Tile), 8-core data-parallel.

Per core (one batch image, L=4096=64x64, C=256):
  qkv = x @ qkv_w; 4 branch attentions on half-channels with strip windows
  (64x8 / 8x64), depthwise-conv LePE added to attention out; concat; proj.

v3 mapping. ACT-exp is the bottleneck (measured: exp[128,1536] PSUM->SBUF,
PE-fed, back-to-back = ~1435 ns => 7.65 us per branch-window, 245 us/core):
  - continuous exp stream: score blocks ([128,512] S^T chunks) are written
    round-robin into 2 PSUM tiles of [128,1536] (3 banks each); each full
    tile fires ONE exp ACTIVATE (N=1536, crossing kc/branch/window
    boundaries). Larger activations amortize the ~350-cyc ACT overhead.
  - PSUM banks (8): sc 2x3 + avden 1 + lepe/aux 1.
  - AV 4-way col-tiled per (kc, head-pair): att (lhsT=V^T slice, M=32) and
    den (lhsT=ones, M=32) at array cols 0/32/64/96; avden [128,512] holds
    [att_h|att_h'|den_h|den_h'] for ONE head-pair; head-pairs sequential.
  - normalize per head-pair: recip [128,512] + one mul [64,512]; cat add
    fused with LePE readout per (hp, window-half).
  - LePE per half-window in a 1-bank PSUM tile: 9 diagonal-weight matmuls
    over a zero-padded flat layout (pitch J+1; shared pad col covers dj=+-1;
    cross-half taps read true neighbor rows from the full vpad).
  - window staging (combo A contiguous copies + V^T transposes) prefetched
    one window ahead so scores never wait on DVE at window boundaries.
  - proj per 128-token chunk: 4 branch matmuls accumulated in PSUM + bias.
"""
import os
import sys

sys.path.insert(0, "/opt/trn_rl_repo")
import numpy as np
import ml_dtypes

import concourse.bacc as bacc
import concourse.mybir as mybir
import concourse.tile as tile
from concourse.bass_utils import run_bass_kernel_spmd
from concourse.masks import make_identity

BF = mybir.dt.bfloat16
F32 = mybir.dt.float32
AF = mybir.ActivationFunctionType
ALU = mybir.AluOpType
SCALE = float(32.0 ** -0.5)

# tap order: (0,0) first so the start=True matmul covers the whole region
TAPS = [(0, 0)] + [(dr, dj) for dr in (-1, 0, 1) for dj in (-1, 0, 1) if (dr, dj) != (0, 0)]

# branch -> (combo, qhalf, kvhalf); combo A = 64x8 windows, B = 8x64
BRANCH = {0: ("A", 0, 0), 1: ("B", 1, 1), 2: ("A", 1, 0), 3: ("B", 0, 1)}

# padded flat window layouts for LePE: (rows, cols, row_pitch, region_base, total)
# pitch = cols+1: single shared pad col between consecutive rows covers dj=+-1.
PAD = {"A": (64, 8, 9, 16, 608), "B": (8, 64, 65, 68, 656)}
# half-window split along rows for the 1-bank lp tiles
NHALF = 2


class ExpStream:
    """Round-robin score blocks into [128,1536] PSUM tiles; one exp per tile."""

    def __init__(self, nc, scps, expsb, width=1536, bufs=13):
        self.nc = nc
        self.bufs = bufs
        self.scps = scps
        self.expsb = expsb
        self.width = width
        self.nslot = width // 512
        self.cur = None
        self.slot = 0
        self.pending = []
        self.out = {}
        self.alloc_count = 0
        self.on_flush = None

    def add_block(self, key, emit_mms):
        if self.cur is None:
            self.cur = self.scps.tile([128, self.width], F32, tag="sc", name="sct")
            self.slot = 0
            self.pending = []
        dst = self.cur[:, 512 * self.slot:512 * self.slot + 512]
        emit_mms(dst)
        self.pending.append(key)
        self.slot += 1
        if self.slot == self.nslot:
            self.flush()

    def flush(self):
        if self.cur is None or self.slot == 0:
            return
        n = 512 * self.slot
        e = self.expsb.tile([128, self.width], BF, tag="exp", name="exp")
        self.alloc_count += 1
        self.nc.scalar.activation(e[:, :n], self.cur[:, :n], AF.Exp, scale=SCALE)
        for i, k in enumerate(self.pending):
            self.out[k] = (e[:, 512 * i:512 * (i + 1)], self.alloc_count)
        self.cur = None
        self.slot = 0
        self.pending = []
        if self.on_flush is not None:
            self.on_flush()


def build(nc, debug=False, repeat=1, dyn_loop=0, with_cbias=True):
    xb = nc.dram_tensor("xb", [4096, 256], BF, kind="ExternalInput").ap()
    qw = nc.dram_tensor("qw", [256, 768], BF, kind="ExternalInput").ap()
    pw = nc.dram_tensor("pw", [512, 256], BF, kind="ExternalInput").ap()
    dg = nc.dram_tensor("dg", [36, 128, 128], BF, kind="ExternalInput").ap()
    pb = nc.dram_tensor("pb", [128, 256], F32, kind="ExternalInput").ap()
    cb = nc.dram_tensor("cb", [128, 4], F32, kind="ExternalInput").ap()
    out_d = nc.dram_tensor("out", [4096, 256], F32, kind="ExternalOutput").ap()
    dbg = {}
    if debug:
        for name, shape in [("d_av", [128, 512]), ("d_rd", [128, 512]),
                            ("d_lep", [128, 656]), ("d_cat", [128, 512]),
                            ("d_vt", [128, 512])]:
            dbg[name] = nc.dram_tensor(name, shape, F32, kind="ExternalOutput").ap()

    with tile.TileContext(nc) as tc:
        with tc.sbuf_pool(name="persist", bufs=1) as ps_pool:
            # ---- constants / weights ----
            ident = ps_pool.tile([128, 128], BF, name="ident")
            make_identity(nc, ident)
            ones32 = ps_pool.tile([128, 32], BF, name="ones32")
            nc.vector.memset(ones32, 1.0)
            ones512 = ps_pool.tile([128, 512], BF, name="ones512")
            nc.vector.memset(ones512, 1.0)
            cbdiag = ps_pool.tile([128, 4 * 128], BF, name="cbdiag")

            qw_t = [ps_pool.tile([128, 768], BF, name=f"qw{i}") for i in range(2)]
            for i in range(2):
                nc.sync.dma_start(qw_t[i], qw[128 * i:128 * (i + 1), :])
            pw_t = [ps_pool.tile([128, 256], BF, name=f"pw{i}") for i in range(4)]
            for i in range(4):
                nc.sync.dma_start(pw_t[i], pw[128 * i:128 * (i + 1), :])
            diag_t = ps_pool.tile([128, 36 * 128], BF, name="diag_t")
            nc.sync.dma_start(diag_t.rearrange("p (t c) -> p t c", t=36),
                              dg.rearrange("t p c -> p t c"))
            pb_t = ps_pool.tile([128, 256], F32, name="pb_t")
            nc.sync.dma_start(pb_t, pb)
            cb_t = ps_pool.tile([128, 4], F32, name="cb_t")
            nc.sync.dma_start(cb_t, cb)
            for _b in range(4):
                nc.vector.tensor_scalar(cbdiag[:, 128 * _b:128 * (_b + 1)], ident,
                                        cb_t[:, _b:_b + 1], None, ALU.mult)

            # ---- persistent activations ----
            q_t = [ps_pool.tile([128, 4096], BF, name=f"q{i}") for i in range(2)]
            k_t = [ps_pool.tile([128, 4096], BF, name=f"k{i}") for i in range(2)]
            v_t = [ps_pool.tile([128, 4096], BF, name=f"v{i}") for i in range(2)]
            cat_t = [ps_pool.tile([128, 4096], BF, name=f"cat{i}") for i in range(4)]

            # persistent zero-padded LePE staging (borders zeroed once,
            # interiors rewritten per window; 2 bufs each for overlap)
            vpads = {}
            for combo, nvp in (("A", 4), ("B", 8)):
                R, J, T, RB, TOT = PAD[combo]
                tiles = [ps_pool.tile([128, TOT], BF, name=f"vpad{combo}{i}")
                         for i in range(nvp)]
                for t in tiles:
                    nc.vector.memset(t, 0.0)
                vpads[combo] = tiles

            def emit_phase0_serial(_rep):
                # serial x^T + QKV (prologue before the steady-state loop)
                with tc.sbuf_pool(name=f"p0sb{_rep}", bufs=1) as p0sb, \
                     tc.tile_pool(name=f"p0ps{_rep}", bufs=4, space="PSUM") as p0ps, \
                     tc.tile_pool(name=f"p0ps2{_rep}", bufs=4, space="PSUM") as p0ps2, \
                     tc.sbuf_pool(name=f"p0in{_rep}", bufs=4) as p0in:
                    xT = [p0sb.tile([128, 4096], BF, name=f"xT{i}") for i in range(2)]
                    for n in range(32):
                        xin = p0in.tile([128, 256], BF, tag="xin", name="xin")
                        nc.sync.dma_start(xin, xb[128 * n:128 * (n + 1), :])
                        for cc in range(2):
                            tp = p0ps.tile([128, 128], BF, tag="tp", name="tp")
                            nc.tensor.transpose(tp, xin[:, 128 * cc:128 * (cc + 1)], ident)
                            nc.vector.tensor_copy(xT[cc][:, 128 * n:128 * (n + 1)], tp)
                    for n in range(8):
                        for m in range(6):
                            qp = p0ps2.tile([128, 512], F32, tag="qp", name="qp")
                            for cc in range(2):
                                nc.tensor.matmul(qp, qw_t[cc][:, 128 * m:128 * (m + 1)],
                                                 xT[cc][:, 512 * n:512 * (n + 1)],
                                                 start=(cc == 0), stop=(cc == 1),
                                                 skip_group_check=True)
                            dst = [q_t, k_t, v_t][m // 2][m % 2]
                            # split evacuation between ACT (idle in phase 0) and DVE
                            if m % 2 == 0:
                                nc.scalar.copy(dst[:, 512 * n:512 * (n + 1)], qp)
                            else:
                                nc.vector.tensor_copy(dst[:, 512 * n:512 * (n + 1)], qp)

            def _emit(_rep):
                # window views (for DVE staging copies only; matmuls need 1-D free)
                def winview(t, combo):
                    if combo == "A":
                        return t.rearrange("c (r w j) -> c w r j", r=64, w=8, j=8)
                    return t.rearrange("c (w i cc) -> c w i cc", w=8, i=8, cc=64)

                # ================= attention =================
                with tc.tile_pool(name=f"scps{_rep}", bufs=2, space="PSUM") as scps, \
                     tc.tile_pool(name=f"avps{_rep}", bufs=1, space="PSUM") as avps, \
                     tc.tile_pool(name=f"auxps{_rep}", bufs=1, space="PSUM") as auxps, \
                     tc.sbuf_pool(name=f"expsb{_rep}", bufs=13) as expsb, \
                     tc.sbuf_pool(name=f"stg{_rep}", bufs=3) as stg, \
                     tc.sbuf_pool(name=f"rdsb{_rep}", bufs=4) as rdsb, \
                     tc.sbuf_pool(name=f"xtsb{_rep}", bufs=8) as xtsb, \
                     tc.sbuf_pool(name=f"vtsb{_rep}", bufs=9) as vtsb, \
                     tc.sbuf_pool(name=f"xinsb{_rep}", bufs=4) as xinsb, \
                     tc.sbuf_pool(name=f"outsb{_rep}", bufs=4) as outsb:

                    es = ExpStream(nc, scps, expsb)
                    # deferred work quanta (cost, closure): popped on exp-tile
                    # flushes under a PE-cost budget so PE's strict-FIFO order
                    # never head-of-line blocks score matmuls behind bulky
                    # AV/LePE work, and ACT never starves
                    workq = []   # branch quanta: AV/norm/LePE (exp-bound)
                    projq = []   # proj + next-iter qkv refill: lowest priority

                    def pop_work():
                        # (cost_us, fn) tuples. workq is emission-ORDER-
                        # SENSITIVE (proj must be emitted after the cat adds
                        # it reads -- dependency direction follows emission
                        # order); projq quanta are order-free (refill/prefetch
                        # whose hazards point the safe way). Budget paces PE
                        # work per exp flush so ACT never starves.
                        budget = 1.5
                        while budget > 0 and workq:
                            cost, fn = workq.pop(0)
                            fn()
                            budget -= cost
                        while budget > 0 and projq:
                            cost, fn = projq.pop(0)
                            fn()
                            budget -= cost

                    def fill_vpad_vt(combo, vwin, vpad, vt_sb):
                        R, J, T, RB, TOT = PAD[combo]
                        nc.vector.tensor_copy(
                            vpad[:, RB:RB + R * T].rearrange(
                                "c (r t) -> c r t", t=T)[:, :, 0:J],
                            vwin.rearrange("c (r j) -> c r j", j=J))
                        vtp = auxps.tile([128, 512], BF, tag="aux", name="vtp")
                        for kc in range(4):
                            nc.tensor.transpose(vtp[:, 128 * kc:128 * (kc + 1)],
                                                vwin[:, 128 * kc:128 * (kc + 1)], ident)
                        nc.vector.tensor_copy(vt_sb, vtp)

                    def stage_window(combo, w):
                        """Stage contiguous q/k/v windows + V^T for an A window
                        (B windows are prebuilt via prep_b quanta)."""
                        assert combo == "A"
                        kvhalf = BRANCH[0][2]
                        kwin = stg.tile([128, 512], BF, tag="kwin", name="kwin")
                        nc.vector.tensor_copy(
                            kwin.rearrange("c (r j) -> c r j", j=8),
                            winview(k_t[kvhalf], "A")[:, w])
                        vwin = stg.tile([128, 512], BF, tag="vwin", name="vwin")
                        nc.vector.tensor_copy(
                            vwin.rearrange("c (r j) -> c r j", j=8),
                            winview(v_t[kvhalf], "A")[:, w])
                        qwin = {}
                        for qh in (0, 1):
                            qt = stg.tile([128, 512], BF, tag=f"qwin{qh}", name="qwin")
                            nc.vector.tensor_copy(
                                qt.rearrange("c (r j) -> c r j", j=8),
                                winview(q_t[qh], "A")[:, w])
                            qwin[qh] = qt
                        vpad = vpads["A"][w % 4]
                        vt_sb = stg.tile([128, 512], BF, tag="vt", name="vt_sb")
                        fill_vpad_vt("A", vwin, vpad, vt_sb)
                        return dict(kwin=kwin, vwin=vwin, qwin=qwin, vpad=vpad,
                                    vt=vt_sb)

                    # B-window staging: direct q/k/v slices; vpad/V^T filled by
                    # low-priority quanta that pop in A-phase slack
                    b_stage = {}

                    def prep_b(w):
                        kvh = BRANCH[1][2]
                        vwin = v_t[kvh][:, 512 * w:512 * (w + 1)]
                        vpad = vpads["B"][w]
                        vt_sb = vtsb.tile([128, 512], BF, tag="bvt", name="bvt")
                        b_stage[w] = dict(
                            kwin=k_t[kvh][:, 512 * w:512 * (w + 1)],
                            vwin=vwin,
                            qwin={qh: q_t[qh][:, 512 * w:512 * (w + 1)]
                                  for qh in (0, 1)},
                            vpad=vpad, vt=vt_sb)
                        projq.append((1.2, lambda: fill_vpad_vt("B", vwin, vpad,
                                                                vt_sb)))

                    def emit_scores(combo, w, br, st):
                        """Feed this branch-window's 16 score blocks into the
                        exp stream; actual AV/norm/LePE runs one bw later."""
                        _, qhalf, kvh = BRANCH[br]
                        qfull = st["qwin"][qhalf]
                        kwin = st["kwin"]

                        def mk_mm(h, kc):
                            def emit(dst):
                                nc.tensor.matmul(
                                    dst,
                                    kwin[32 * h:32 * (h + 1), 128 * kc:128 * (kc + 1)],
                                    qfull[32 * h:32 * (h + 1), :],
                                    start=True, stop=True,
                                    tile_position=(32 * h, 0))
                            return emit

                        for kc in range(4):
                            for h in range(4):
                                es.add_block((br, w, h, kc), mk_mm(h, kc))
                        return (combo, w, br, st)

                    def consume(ctx):
                        combo, w, br, st = ctx
                        R, J, T, RB, TOT = PAD[combo]
                        vt_sb, vpad = st["vt"], st["vpad"]
                        is_dbg = debug and br == 0 and w == 0
                        span = R * T // NHALF
                        rh = R // NHALF

                        def lepe_half(half):
                            base = RB + span * half
                            lp = auxps.tile([128, RB + span], F32, tag="aux", name="lp")
                            for t, (dr, dj) in enumerate(TAPS):
                                delta = T * dr + dj
                                dmat = diag_t[:, (br * 9 + t) * 128:(br * 9 + t + 1) * 128]
                                nc.tensor.matmul(
                                    lp[:, RB:RB + span],
                                    dmat,
                                    vpad[:, base + delta:base + span + delta],
                                    start=(t == 0),
                                    stop=(not with_cbias and t == 8),
                                    skip_group_check=True)
                            if with_cbias:
                                nc.tensor.matmul(
                                    lp[:, RB:RB + span],
                                    cbdiag[:, 128 * br:128 * (br + 1)],
                                    ones512[:, 0:span],
                                    start=False, stop=True, skip_group_check=True)
                            return lp

                        # --- AV + den for one head-pair (1-bank avden) ---
                        rds = []

                        def av_norm(hp):
                            avden = avps.tile([128, 512], F32, tag="av", name="avden")
                            for kc in range(4):
                                for hs in range(2):
                                    h = 2 * hp + hs
                                    ecols, eidx = es.out[(br, w, h, kc)]
                                    assert es.alloc_count - eidx < es.bufs - 1, (
                                        f"exp ring too shallow: read lag "
                                        f"{es.alloc_count - eidx} vs bufs {es.bufs}")
                                    nc.tensor.matmul(
                                        avden[32 * hs:32 * hs + 32, :],
                                        vt_sb[:, 128 * kc + 32 * h:128 * kc + 32 * h + 32],
                                        ecols,
                                        start=(kc == 0), stop=(kc == 3),
                                        tile_position=(0, 32 * hs),
                                        skip_group_check=True)
                                    nc.tensor.matmul(
                                        avden[64 + 32 * hs:64 + 32 * hs + 32, :],
                                        ones32,
                                        ecols,
                                        start=(kc == 0), stop=(kc == 3),
                                        tile_position=(0, 64 + 32 * hs),
                                        skip_group_check=True)
                            # normalize: rd[64:128]=1/den (full-tile custom op
                            # reads base partition 0); rd[0:64]=att*recip
                            rd = rdsb.tile([128, 512], F32, tag="rd", name="rd")
                            nc.vector.reciprocal_approx_fast(rd, avden)
                            nc.vector.tensor_mul(rd[0:64, :], avden[0:64, :],
                                                 rd[64:128, :])
                            rds.append(rd)
                            if is_dbg and hp == 0:
                                for nm, src in [("d_av", avden), ("d_rd", rd)]:
                                    dt_ = stg.tile([128, 512], F32, tag="dbg" + nm, name="dT")
                                    nc.vector.tensor_copy(dt_, src)
                                    nc.sync.dma_start(dbg[nm], dt_)

                        # --- cat = att*rd + lepe for one window-half; lp
                        # halves sequential on the single aux bank ---
                        catw = winview(cat_t[br], combo)[:, w]       # [c, R, J]

                        def lepe_add_half(half):
                            lp = lepe_half(half)
                            lpv = lp[:, RB:RB + span].rearrange(
                                "c (r t) -> c r t", t=T)[:, :, 0:J]
                            for hp in range(2):
                                tview = rds[hp][0:64, :].rearrange(
                                    "c (r j) -> c r j", j=J)
                                nc.vector.tensor_add(
                                    catw[64 * hp:64 * hp + 64,
                                         rh * half:rh * (half + 1)],
                                    tview[:, rh * half:rh * (half + 1)],
                                    lpv[64 * hp:64 * hp + 64])
                            if is_dbg:
                                dl = stg.tile([128, 328], F32, tag="dbglep", name="dlep")
                                nc.vector.tensor_copy(dl[:, :RB + span], lp)
                                nc.sync.dma_start(
                                    dbg["d_lep"][:, 328 * half:328 * half + RB + span],
                                    dl[:, :RB + span])
                            if is_dbg and half == NHALF - 1:
                                dc = stg.tile([128, 512], F32, tag="dbgcat", name="dcat")
                                nc.vector.tensor_copy(
                                    dc.rearrange("c (a b) -> c a b", a=R, b=J), catw)
                                nc.sync.dma_start(dbg["d_cat"], dc)
                                dv = stg.tile([128, 512], F32, tag="dbgvt", name="dvt")
                                nc.vector.tensor_copy(dv, vt_sb)
                                nc.sync.dma_start(dbg["d_vt"], dv)

                        workq.append((0.9, lambda: av_norm(0)))
                        workq.append((0.9, lambda: av_norm(1)))
                        workq.append((1.3, lambda: lepe_add_half(0)))
                        workq.append((1.3, lambda: lepe_add_half(1)))

                    # window order: all B windows, then all A windows; staging
                    # prefetched one window ahead; AV/norm/LePE consumption
                    # deferred one branch-window so its exps have flushed
                    def proj_chunk(n):
                        pp = auxps.tile([128, 256], F32, tag="aux", name="pp")
                        for b2 in range(4):
                            nc.tensor.matmul(pp, cat_t[b2][:, 128 * n:128 * (n + 1)],
                                             pw_t[b2], start=(b2 == 0), stop=(b2 == 3),
                                             skip_group_check=True)
                        osb = outsb.tile([128, 256], F32, tag="out", name="osb")
                        nc.vector.tensor_add(osb, pp, pb_t)
                        nc.sync.dma_start(out_d[128 * n:128 * (n + 1), :], osb)

                    # next-iteration qkv refill for token chunk w, split:
                    # x^T transposes have no attention deps (pop in A-phase
                    # slack); the QKV matmuls overwrite q/k/v chunk w, legal
                    # once B-window w's direct reads are emitted (Tile WAR)
                    xtc_store = {}

                    def xpose_quanta(w):
                        xtc = [xtsb.tile([128, 512], BF, tag=f"xt{cc}", name="xtc")
                               for cc in range(2)]
                        xtc_store[w] = xtc

                        def xpose(t):
                            xin = xinsb.tile([128, 256], BF, tag="xin", name="xin")
                            nc.sync.dma_start(xin, xb[512 * w + 128 * t:
                                                      512 * w + 128 * (t + 1), :])
                            tp = auxps.tile([128, 256], BF, tag="aux", name="tp")
                            for cc in range(2):
                                nc.tensor.transpose(tp[:, 128 * cc:128 * (cc + 1)],
                                                    xin[:, 128 * cc:128 * (cc + 1)],
                                                    ident)
                            for cc in range(2):
                                nc.vector.tensor_copy(
                                    xtc[cc][:, 128 * t:128 * (t + 1)],
                                    tp[:, 128 * cc:128 * (cc + 1)])

                        return [(0.75, lambda t=t: xpose(t)) for t in range(4)]

                    def qkv_quanta(w):
                        xtc = xtc_store[w]

                        def qkv_mm(m):
                            qp = avps.tile([128, 512], F32, tag="av", name="qp")
                            for cc in range(2):
                                nc.tensor.matmul(qp, qw_t[cc][:, 128 * m:128 * (m + 1)],
                                                 xtc[cc],
                                                 start=(cc == 0), stop=(cc == 1),
                                                 skip_group_check=True)
                            dst = [q_t, k_t, v_t][m // 2][m % 2]
                            nc.vector.tensor_copy(dst[:, 512 * w:512 * (w + 1)], qp)

                        return [(0.7, lambda m=m: qkv_mm(m)) for m in range(6)]

                    # A windows first (need full qkv anyway), B windows last:
                    # a B window owns token chunks 4w..4w+3, so proj for those
                    # chunks + the chunk's qkv refill stream in right behind it
                    order = [("A", w) for w in range(8)] + [("B", w) for w in range(8)]
                    es.on_flush = pop_work
                    staged = {}
                    staged[order[0]] = stage_window(*order[0])
                    for i, (combo, w) in enumerate(order):
                        if i + 1 < len(order):
                            nxt = order[i + 1]
                            if nxt[0] == "A":
                                staged[nxt] = stage_window(*nxt)
                        st = staged.pop((combo, w)) if combo == "A" else b_stage[w]
                        branches = (0, 2) if combo == "A" else (1, 3)
                        for br in branches:
                            consume(emit_scores(combo, w, br, st))
                        if combo == "A":
                            if 1 <= w <= 4:
                                projq.extend(xpose_quanta(2 * (w - 1)))
                                projq.extend(xpose_quanta(2 * (w - 1) + 1))
                            elif w == 5:
                                for bw in range(4):
                                    prep_b(bw)
                            elif w == 6:
                                for bw in range(4, 8):
                                    prep_b(bw)
                        else:
                            for n in range(4 * w, 4 * w + 4):
                                workq.append((0.5, lambda n=n: proj_chunk(n)))
                            projq.extend(qkv_quanta(w))
                    es.flush()
                    es.on_flush = None
                    while workq:
                        workq.pop(0)[1]()
                    while projq:
                        projq.pop(0)[1]()

            emit_phase0_serial(0)
            if dyn_loop:
                with tc.For_i(0, dyn_loop, 1):
                    _emit(0)
            else:
                for _rep in range(repeat):
                    _emit(_rep)

    return nc


_CACHE = {}


def _get_nc(debug=False, repeat=1, dyn_loop=0, with_cbias=True):
    key = (bool(debug), repeat, dyn_loop, with_cbias)
    if key not in _CACHE:
        nc = bacc.Bacc("TRN2", target_bir_lowering=False, debug=False)
        build(nc, debug=debug, repeat=repeat, dyn_loop=dyn_loop, with_cbias=with_cbias)
        nc.compile()
        _CACHE[key] = nc
    return _CACHE[key]


def prep_inputs(x, qkv_w, proj_w, proj_b, conv_ws, conv_bs):
    x = np.asarray(x)
    B = x.shape[0]
    xb = x.astype(ml_dtypes.bfloat16)
    qwb = np.asarray(qkv_w).astype(ml_dtypes.bfloat16)
    pwb = np.asarray(proj_w).astype(ml_dtypes.bfloat16)
    w9 = np.asarray(conv_ws).reshape(4, 128, 9).astype(np.float32)
    dgn = np.zeros((36, 128, 128), np.float32)
    idx = np.arange(128)
    for br in range(4):
        for t, (dr, dj) in enumerate(TAPS):
            dgn[br * 9 + t, idx, idx] = w9[br, :, (dr + 1) * 3 + (dj + 1)]
    dgn = dgn.astype(ml_dtypes.bfloat16)
    pbb = np.tile(np.asarray(proj_b, np.float32)[None, :], (128, 1))
    cbt = np.ascontiguousarray(np.asarray(conv_bs, np.float32).T)
    shared = {"qw": qwb, "pw": pwb, "dg": dgn, "pb": pbb, "cb": cbt}
    return [dict(shared, xb=np.ascontiguousarray(xb[b])) for b in range(B)]


def kernel(x, qkv_w, proj_w, proj_b, conv_ws, conv_bs, _debug=False, _trace=False):
    wcb = bool(np.any(np.asarray(conv_bs)))
    nc = _get_nc(debug=_debug, with_cbias=wcb)
    in_maps = prep_inputs(x, qkv_w, proj_w, proj_b, conv_ws, conv_bs)
    res = run_bass_kernel_spmd(nc, in_maps, core_ids=list(range(len(in_maps))),
                               trace=_trace)
    out = np.stack([r["out"] for r in res.results]).astype(np.float32)
    if _debug or _trace:
        kernel.last_results = res
    return out


# revision 38
# speedup vs baseline: 1.0123x; 1.0123x over previous
"""CSWin-style cross-attention block for Trainium2 (Bass/Tile), 8-core data-parallel.

Per core (one batch image, L=4096=64x64, C=256):
  qkv = x @ qkv_w; 4 branch attentions on half-channels with strip windows
  (64x8 / 8x64), depthwise-conv LePE added to attention out; concat; proj.

v3 mapping. ACT-exp is the bottleneck (measured: exp[128,1536] PSUM->SBUF,
PE-fed, back-to-back = ~1435 ns => 7.65 us per branch-window, 245 us/core):
  - continuous exp stream: score blocks ([128,512] S^T chunks) are written
    round-robin into 2 PSUM tiles of [128,1536] (3 banks each); each full
    tile fires ONE exp ACTIVATE (N=1536, crossing kc/branch/window
    boundaries). Larger activations amortize the ~350-cyc ACT overhead.
  - PSUM banks (8): sc 2x3 + avden 1 + lepe/aux 1.
  - AV 4-way col-tiled per (kc, head-pair): att (lhsT=V^T slice, M=32) and
    den (lhsT=ones, M=32) at array cols 0/32/64/96; avden [128,512] holds
    [att_h|att_h'|den_h|den_h'] for ONE head-pair; head-pairs sequential.
  - normalize per head-pair: recip [128,512] + one mul [64,512]; cat add
    fused with LePE readout per (hp, window-half).
  - LePE per half-window in a 1-bank PSUM tile: 9 diagonal-weight matmuls
    over a zero-padded flat layout (pitch J+1; shared pad col covers dj=+-1;
    cross-half taps read true neighbor rows from the full vpad).
  - window staging (combo A contiguous copies + V^T transposes) prefetched
    one window ahead so scores never wait on DVE at window boundaries.
  - proj per 128-token chunk: 4 branch matmuls accumulated in PSUM + bias.
"""
import os
import sys

sys.path.insert(0, "/opt/trn_rl_repo")
import numpy as np
import ml_dtypes

import concourse.bacc as bacc
import concourse.mybir as mybir
import concourse.tile as tile
from concourse.bass_utils import run_bass_kernel_spmd
from concourse.masks import make_identity

BF = mybir.dt.bfloat16
F32 = mybir.dt.float32
AF = mybir.ActivationFunctionType
ALU = mybir.AluOpType
SCALE = float(32.0 ** -0.5)

# tap order: (0,0) first so the start=True matmul covers the whole region
TAPS = [(0, 0)] + [(dr, dj) for dr in (-1, 0, 1) for dj in (-1, 0, 1) if (dr, dj) != (0, 0)]

# branch -> (combo, qhalf, kvhalf); combo A = 64x8 windows, B = 8x64
BRANCH = {0: ("A", 0, 0), 1: ("B", 1, 1), 2: ("A", 1, 0), 3: ("B", 0, 1)}

# padded flat window layouts for LePE: (rows, cols, row_pitch, region_base, total)
# pitch = cols+1: single shared pad col between consecutive rows covers dj=+-1.
PAD = {"A": (64, 8, 9, 16, 608), "B": (8, 64, 65, 68, 656)}
# half-window split along rows for the 1-bank lp tiles
NHALF = 2


class ExpStream:
    """Round-robin score blocks into [128,1536] PSUM tiles; one exp per tile."""

    def __init__(self, nc, scps, expsb, width=1536, bufs=18):
        self.nc = nc
        self.bufs = bufs
        self.scps = scps
        self.expsb = expsb
        self.width = width
        self.nslot = width // 512
        self.cur = None
        self.slot = 0
        self.pending = []
        self.out = {}
        self.alloc_count = 0
        self.on_flush = None

    def add_block(self, key, emit_mms):
        if self.cur is None:
            self.cur = self.scps.tile([128, self.width], F32, tag="sc", name="sct")
            self.slot = 0
            self.pending = []
        dst = self.cur[:, 512 * self.slot:512 * self.slot + 512]
        emit_mms(dst)
        self.pending.append(key)
        self.slot += 1
        if self.slot == self.nslot:
            self.flush()

    def flush(self):
        if self.cur is None or self.slot == 0:
            return
        n = 512 * self.slot
        e = self.expsb.tile([128, self.width], BF, tag="exp", name="exp")
        self.alloc_count += 1
        self.nc.scalar.activation(e[:, :n], self.cur[:, :n], AF.Exp, scale=SCALE)
        for i, k in enumerate(self.pending):
            self.out[k] = (e[:, 512 * i:512 * (i + 1)], self.alloc_count)
        self.cur = None
        self.slot = 0
        self.pending = []
        if self.on_flush is not None:
            self.on_flush()


def build(nc, debug=False, repeat=1, dyn_loop=0, with_cbias=True):
    xb = nc.dram_tensor("xb", [4096, 256], BF, kind="ExternalInput").ap()
    qw = nc.dram_tensor("qw", [256, 768], BF, kind="ExternalInput").ap()
    pw = nc.dram_tensor("pw", [512, 256], BF, kind="ExternalInput").ap()
    dg = nc.dram_tensor("dg", [36, 128, 128], BF, kind="ExternalInput").ap()
    pb = nc.dram_tensor("pb", [128, 256], F32, kind="ExternalInput").ap()
    cb = nc.dram_tensor("cb", [128, 4], F32, kind="ExternalInput").ap()
    out_d = nc.dram_tensor("out", [4096, 256], F32, kind="ExternalOutput").ap()
    dbg = {}
    if debug:
        for name, shape in [("d_av", [128, 512]), ("d_rd", [128, 512]),
                            ("d_lep", [128, 656]), ("d_cat", [128, 512]),
                            ("d_vt", [128, 512])]:
            dbg[name] = nc.dram_tensor(name, shape, F32, kind="ExternalOutput").ap()

    with tile.TileContext(nc) as tc:
        with tc.sbuf_pool(name="persist", bufs=1) as ps_pool:
            # ---- constants / weights ----
            ident = ps_pool.tile([128, 128], BF, name="ident")
            make_identity(nc, ident)
            ones32 = ps_pool.tile([128, 32], BF, name="ones32")
            nc.vector.memset(ones32, 1.0)
            ones512 = ps_pool.tile([128, 512], BF, name="ones512")
            nc.vector.memset(ones512, 1.0)
            cbdiag = ps_pool.tile([128, 4 * 128], BF, name="cbdiag")

            qw_t = [ps_pool.tile([128, 768], BF, name=f"qw{i}") for i in range(2)]
            for i in range(2):
                nc.sync.dma_start(qw_t[i], qw[128 * i:128 * (i + 1), :])
            pw_t = [ps_pool.tile([128, 256], BF, name=f"pw{i}") for i in range(4)]
            for i in range(4):
                nc.sync.dma_start(pw_t[i], pw[128 * i:128 * (i + 1), :])
            diag_t = ps_pool.tile([128, 36 * 128], BF, name="diag_t")
            nc.sync.dma_start(diag_t.rearrange("p (t c) -> p t c", t=36),
                              dg.rearrange("t p c -> p t c"))
            pb_t = ps_pool.tile([128, 256], F32, name="pb_t")
            nc.sync.dma_start(pb_t, pb)
            cb_t = ps_pool.tile([128, 4], F32, name="cb_t")
            nc.sync.dma_start(cb_t, cb)
            for _b in range(4):
                nc.vector.tensor_scalar(cbdiag[:, 128 * _b:128 * (_b + 1)], ident,
                                        cb_t[:, _b:_b + 1], None, ALU.mult)

            # ---- persistent activations ----
            q_t = [ps_pool.tile([128, 4096], BF, name=f"q{i}") for i in range(2)]
            k_t = [ps_pool.tile([128, 4096], BF, name=f"k{i}") for i in range(2)]
            v_t = [ps_pool.tile([128, 4096], BF, name=f"v{i}") for i in range(2)]
            cat_t = [ps_pool.tile([128, 4096], BF, name=f"cat{i}") for i in range(4)]

            # persistent zero-padded LePE staging (borders zeroed once,
            # interiors rewritten per window; 2 bufs each for overlap)
            vpads = {}
            for combo in ("A", "B"):
                R, J, T, RB, TOT = PAD[combo]
                tiles = [ps_pool.tile([128, TOT], BF, name=f"vpad{combo}{i}")
                         for i in range(4)]
                for t in tiles:
                    nc.vector.memset(t, 0.0)
                vpads[combo] = tiles

            def _emit(_rep):
                # ================= phase 0: x^T + QKV =================
                with tc.sbuf_pool(name=f"p0sb{_rep}", bufs=1) as p0sb, \
                     tc.tile_pool(name=f"p0ps{_rep}", bufs=4, space="PSUM") as p0ps, \
                     tc.tile_pool(name=f"p0ps2{_rep}", bufs=4, space="PSUM") as p0ps2, \
                     tc.sbuf_pool(name=f"p0in{_rep}", bufs=4) as p0in:
                    xT = [p0sb.tile([128, 4096], BF, name=f"xT{i}") for i in range(2)]
                    for n in range(32):
                        xin = p0in.tile([128, 256], BF, tag="xin", name="xin")
                        nc.sync.dma_start(xin, xb[128 * n:128 * (n + 1), :])
                        for cc in range(2):
                            tp = p0ps.tile([128, 128], BF, tag="tp", name="tp")
                            nc.tensor.transpose(tp, xin[:, 128 * cc:128 * (cc + 1)], ident)
                            nc.vector.tensor_copy(xT[cc][:, 128 * n:128 * (n + 1)], tp)
                    for n in range(8):
                        for m in range(6):
                            qp = p0ps2.tile([128, 512], F32, tag="qp", name="qp")
                            for cc in range(2):
                                nc.tensor.matmul(qp, qw_t[cc][:, 128 * m:128 * (m + 1)],
                                                 xT[cc][:, 512 * n:512 * (n + 1)],
                                                 start=(cc == 0), stop=(cc == 1),
                                                 skip_group_check=True)
                            dst = [q_t, k_t, v_t][m // 2][m % 2]
                            # split evacuation between ACT (idle in phase 0) and DVE
                            if m % 2 == 0:
                                nc.scalar.copy(dst[:, 512 * n:512 * (n + 1)], qp)
                            else:
                                nc.vector.tensor_copy(dst[:, 512 * n:512 * (n + 1)], qp)

                # window views (for DVE staging copies only; matmuls need 1-D free)
                def winview(t, combo):
                    if combo == "A":
                        return t.rearrange("c (r w j) -> c w r j", r=64, w=8, j=8)
                    return t.rearrange("c (w i cc) -> c w i cc", w=8, i=8, cc=64)

                # ================= attention =================
                with tc.tile_pool(name=f"scps{_rep}", bufs=2, space="PSUM") as scps, \
                     tc.tile_pool(name=f"avps{_rep}", bufs=1, space="PSUM") as avps, \
                     tc.tile_pool(name=f"auxps{_rep}", bufs=1, space="PSUM") as auxps, \
                     tc.sbuf_pool(name=f"expsb{_rep}", bufs=18) as expsb, \
                     tc.sbuf_pool(name=f"stg{_rep}", bufs=4) as stg, \
                     tc.sbuf_pool(name=f"rdsb{_rep}", bufs=4) as rdsb, \
                     tc.sbuf_pool(name=f"outsb{_rep}", bufs=4) as outsb:

                    es = ExpStream(nc, scps, expsb)
                    # deferred work quanta (closures), popped one per exp-tile
                    # flush so PE's strict-FIFO order never head-of-line
                    # blocks score matmuls behind bulky AV/LePE work
                    workq = []   # branch quanta: AV/norm/LePE (exp-bound)
                    projq = []   # proj chunks: no exp deps, lowest priority

                    def pop_work():
                        if workq:
                            workq.pop(0)()
                        elif projq:
                            projq.pop(0)()

                    def stage_window(combo, w):
                        """Stage contiguous q/k/v windows + V^T for (combo, w)."""
                        R, J, T, RB, TOT = PAD[combo]
                        branches = (0, 2) if combo == "A" else (1, 3)
                        kvhalf = BRANCH[branches[0]][2]
                        if combo == "A":
                            kwin = stg.tile([128, 512], BF, tag="kwin", name="kwin")
                            nc.vector.tensor_copy(
                                kwin.rearrange("c (r j) -> c r j", j=8),
                                winview(k_t[kvhalf], "A")[:, w])
                            vwin = stg.tile([128, 512], BF, tag="vwin", name="vwin")
                            nc.vector.tensor_copy(
                                vwin.rearrange("c (r j) -> c r j", j=8),
                                winview(v_t[kvhalf], "A")[:, w])
                            qwin = {}
                            for qh in (0, 1):
                                qt = stg.tile([128, 512], BF, tag=f"qwin{qh}", name="qwin")
                                nc.vector.tensor_copy(
                                    qt.rearrange("c (r j) -> c r j", j=8),
                                    winview(q_t[qh], "A")[:, w])
                                qwin[qh] = qt
                        else:
                            kwin = k_t[kvhalf][:, 512 * w:512 * (w + 1)]
                            vwin = v_t[kvhalf][:, 512 * w:512 * (w + 1)]
                            qwin = {qh: q_t[qh][:, 512 * w:512 * (w + 1)]
                                    for qh in (0, 1)}
                        # zero-padded v window for LePE (interior only)
                        vpad = vpads[combo][w % 4]
                        nc.vector.tensor_copy(
                            vpad[:, RB:RB + R * T].rearrange(
                                "c (r t) -> c r t", t=T)[:, :, 0:J],
                            vwin.rearrange("c (r j) -> c r j", j=J))
                        # V^T: 4 PE transposes (aux psum bank) + one copy
                        vtp = auxps.tile([128, 512], BF, tag="aux", name="vtp")
                        for kc in range(4):
                            nc.tensor.transpose(vtp[:, 128 * kc:128 * (kc + 1)],
                                                vwin[:, 128 * kc:128 * (kc + 1)], ident)
                        vt_sb = stg.tile([128, 512], BF, tag="vt", name="vt_sb")
                        nc.vector.tensor_copy(vt_sb, vtp)
                        return dict(kwin=kwin, vwin=vwin, qwin=qwin, vpad=vpad,
                                    vt=vt_sb)

                    def emit_scores(combo, w, br, st):
                        """Feed this branch-window's 16 score blocks into the
                        exp stream; actual AV/norm/LePE runs one bw later."""
                        _, qhalf, kvh = BRANCH[br]
                        qfull = st["qwin"][qhalf]
                        kwin = st["kwin"]

                        def mk_mm(h, kc):
                            def emit(dst):
                                nc.tensor.matmul(
                                    dst,
                                    kwin[32 * h:32 * (h + 1), 128 * kc:128 * (kc + 1)],
                                    qfull[32 * h:32 * (h + 1), :],
                                    start=True, stop=True,
                                    tile_position=(32 * h, 0))
                            return emit

                        for kc in range(4):
                            for h in range(4):
                                es.add_block((br, w, h, kc), mk_mm(h, kc))
                        return (combo, w, br, st)

                    def consume(ctx):
                        combo, w, br, st = ctx
                        R, J, T, RB, TOT = PAD[combo]
                        vt_sb, vpad = st["vt"], st["vpad"]
                        is_dbg = debug and br == 0 and w == 0
                        span = R * T // NHALF
                        rh = R // NHALF

                        def lepe_half(half):
                            base = RB + span * half
                            lp = auxps.tile([128, RB + span], F32, tag="aux", name="lp")
                            for t, (dr, dj) in enumerate(TAPS):
                                delta = T * dr + dj
                                dmat = diag_t[:, (br * 9 + t) * 128:(br * 9 + t + 1) * 128]
                                nc.tensor.matmul(
                                    lp[:, RB:RB + span],
                                    dmat,
                                    vpad[:, base + delta:base + span + delta],
                                    start=(t == 0),
                                    stop=(not with_cbias and t == 8),
                                    skip_group_check=True)
                            if with_cbias:
                                nc.tensor.matmul(
                                    lp[:, RB:RB + span],
                                    cbdiag[:, 128 * br:128 * (br + 1)],
                                    ones512[:, 0:span],
                                    start=False, stop=True, skip_group_check=True)
                            return lp

                        # --- AV + den for one head-pair (1-bank avden) ---
                        rds = []

                        def av_norm(hp):
                            avden = avps.tile([128, 512], F32, tag="av", name="avden")
                            for kc in range(4):
                                for hs in range(2):
                                    h = 2 * hp + hs
                                    ecols, eidx = es.out[(br, w, h, kc)]
                                    assert es.alloc_count - eidx < es.bufs - 1, (
                                        f"exp ring too shallow: read lag "
                                        f"{es.alloc_count - eidx} vs bufs {es.bufs}")
                                    nc.tensor.matmul(
                                        avden[32 * hs:32 * hs + 32, :],
                                        vt_sb[:, 128 * kc + 32 * h:128 * kc + 32 * h + 32],
                                        ecols,
                                        start=(kc == 0), stop=(kc == 3),
                                        tile_position=(0, 32 * hs),
                                        skip_group_check=True)
                                    nc.tensor.matmul(
                                        avden[64 + 32 * hs:64 + 32 * hs + 32, :],
                                        ones32,
                                        ecols,
                                        start=(kc == 0), stop=(kc == 3),
                                        tile_position=(0, 64 + 32 * hs),
                                        skip_group_check=True)
                            # normalize: rd[64:128]=1/den (full-tile custom op
                            # reads base partition 0); rd[0:64]=att*recip
                            rd = rdsb.tile([128, 512], F32, tag="rd", name="rd")
                            nc.vector.reciprocal_approx_fast(rd, avden)
                            nc.vector.tensor_mul(rd[0:64, :], avden[0:64, :],
                                                 rd[64:128, :])
                            rds.append(rd)
                            if is_dbg and hp == 0:
                                for nm, src in [("d_av", avden), ("d_rd", rd)]:
                                    dt_ = stg.tile([128, 512], F32, tag="dbg" + nm, name="dT")
                                    nc.vector.tensor_copy(dt_, src)
                                    nc.sync.dma_start(dbg[nm], dt_)

                        # --- cat = att*rd + lepe for one window-half; lp
                        # halves sequential on the single aux bank ---
                        catw = winview(cat_t[br], combo)[:, w]       # [c, R, J]

                        def lepe_add_half(half):
                            lp = lepe_half(half)
                            lpv = lp[:, RB:RB + span].rearrange(
                                "c (r t) -> c r t", t=T)[:, :, 0:J]
                            for hp in range(2):
                                tview = rds[hp][0:64, :].rearrange(
                                    "c (r j) -> c r j", j=J)
                                nc.vector.tensor_add(
                                    catw[64 * hp:64 * hp + 64,
                                         rh * half:rh * (half + 1)],
                                    tview[:, rh * half:rh * (half + 1)],
                                    lpv[64 * hp:64 * hp + 64])
                            if is_dbg:
                                dl = stg.tile([128, 328], F32, tag="dbglep", name="dlep")
                                nc.vector.tensor_copy(dl[:, :RB + span], lp)
                                nc.sync.dma_start(
                                    dbg["d_lep"][:, 328 * half:328 * half + RB + span],
                                    dl[:, :RB + span])
                            if is_dbg and half == NHALF - 1:
                                dc = stg.tile([128, 512], F32, tag="dbgcat", name="dcat")
                                nc.vector.tensor_copy(
                                    dc.rearrange("c (a b) -> c a b", a=R, b=J), catw)
                                nc.sync.dma_start(dbg["d_cat"], dc)
                                dv = stg.tile([128, 512], F32, tag="dbgvt", name="dvt")
                                nc.vector.tensor_copy(dv, vt_sb)
                                nc.sync.dma_start(dbg["d_vt"], dv)

                        workq.append(lambda: av_norm(0))
                        workq.append(lambda: av_norm(1))
                        workq.append(lambda: lepe_add_half(0))
                        workq.append(lambda: lepe_add_half(1))

                    # window order: all B windows, then all A windows; staging
                    # prefetched one window ahead; AV/norm/LePE consumption
                    # deferred one branch-window so its exps have flushed
                    def proj_chunk(n):
                        pp = auxps.tile([128, 256], F32, tag="aux", name="pp")
                        for b2 in range(4):
                            nc.tensor.matmul(pp, cat_t[b2][:, 128 * n:128 * (n + 1)],
                                             pw_t[b2], start=(b2 == 0), stop=(b2 == 3),
                                             skip_group_check=True)
                        osb = outsb.tile([128, 256], F32, tag="out", name="osb")
                        nc.vector.tensor_add(osb, pp, pb_t)
                        nc.sync.dma_start(out_d[128 * n:128 * (n + 1), :], osb)

                    # A windows first (need full qkv anyway), B windows last:
                    # a B window owns token chunks 4w..4w+3, so proj for those
                    # chunks streams into the work queue right behind it
                    order = [("A", w) for w in range(8)] + [("B", w) for w in range(8)]
                    es.on_flush = pop_work
                    staged = {}
                    staged[order[0]] = stage_window(*order[0])
                    for i, (combo, w) in enumerate(order):
                        if i + 1 < len(order):
                            staged[order[i + 1]] = stage_window(*order[i + 1])
                        st = staged.pop((combo, w))
                        branches = (0, 2) if combo == "A" else (1, 3)
                        for br in branches:
                            consume(emit_scores(combo, w, br, st))
                        if combo == "B":
                            for n in range(4 * w, 4 * w + 4):
                                projq.append(lambda n=n: proj_chunk(n))
                    es.flush()
                    es.on_flush = None
                    while workq:
                        workq.pop(0)()
                    while projq:
                        projq.pop(0)()

            if dyn_loop:
                with tc.For_i(0, dyn_loop, 1):
                    _emit(0)
            else:
                for _rep in range(repeat):
                    _emit(_rep)

    return nc


_CACHE = {}


def _get_nc(debug=False, repeat=1, dyn_loop=0, with_cbias=True):
    key = (bool(debug), repeat, dyn_loop, with_cbias)
    if key not in _CACHE:
        nc = bacc.Bacc("TRN2", target_bir_lowering=False, debug=False)
        build(nc, debug=debug, repeat=repeat, dyn_loop=dyn_loop, with_cbias=with_cbias)
        nc.compile()
        _CACHE[key] = nc
    return _CACHE[key]


def prep_inputs(x, qkv_w, proj_w, proj_b, conv_ws, conv_bs):
    x = np.asarray(x)
    B = x.shape[0]
    xb = x.astype(ml_dtypes.bfloat16)
    qwb = np.asarray(qkv_w).astype(ml_dtypes.bfloat16)
    pwb = np.asarray(proj_w).astype(ml_dtypes.bfloat16)
    w9 = np.asarray(conv_ws).reshape(4, 128, 9).astype(np.float32)
    dgn = np.zeros((36, 128, 128), np.float32)
    idx = np.arange(128)
    for br in range(4):
        for t, (dr, dj) in enumerate(TAPS):
            dgn[br * 9 + t, idx, idx] = w9[br, :, (dr + 1) * 3 + (dj + 1)]
    dgn = dgn.astype(ml_dtypes.bfloat16)
    pbb = np.tile(np.asarray(proj_b, np.float32)[None, :], (128, 1))
    cbt = np.ascontiguousarray(np.asarray(conv_bs, np.float32).T)
    shared = {"qw": qwb, "pw": pwb, "dg": dgn, "pb": pbb, "cb": cbt}
    return [dict(shared, xb=np.ascontiguousarray(xb[b])) for b in range(B)]


def kernel(x, qkv_w, proj_w, proj_b, conv_ws, conv_bs, _debug=False, _trace=False):
    wcb = bool(np.any(np.asarray(conv_bs)))
    nc = _get_nc(debug=_debug, with_cbias=wcb)
    in_maps = prep_inputs(x, qkv_w, proj_w, proj_b, conv_ws, conv_bs)
    res = run_bass_kernel_spmd(nc, in_maps, core_ids=list(range(len(in_maps))),
                               trace=_trace)
    out = np.stack([r["out"] for r in res.results]).astype(np.float32)
    if _debug or _trace:
        kernel.last_results = res
    return out
